# revision 1
# baseline (speedup 1.0000x reference)
"""Trainium2 Bass kernel for nn_EnterpriseNeuralMemory (scatter_memory).

Sharding: data-parallel over batch — 8 batch elements, one per NeuronCore.
No collectives needed (router mean is per-batch-element and chunk pooling is
chunk-local).

Per-core algorithm (batch element b, all layouts transposed = [feature, pos]):
  logitsT = attn_w.T @ x.T          (PE, bf16, 16 pos-tiles of 512)
  E^T = exp(logitsT)                (ACT, PSUM->SBUF bf16)
  P^T = x^T * E^T                   (DVE bf16 2x)
  Z = segsum64(E^T), N = segsum64(P^T)  (DVE binary trees, bf16->f32)
  m = segsum64(x)/64 via block-ones matmul on PE (natural layout x)
  conv_pool  = W0@(m+u/64) + W1@m + W2@(m+v/64) + conv_b
               (boundary algebra: u/v from strided firsts/lasts columns)
  router: mean of chunk-first tokens -> 2-layer MLP -> softmax(3)
  out = r0*m + r1*(N/Z) + r2*conv_pool
"""

import numpy as np
import ml_dtypes

BF16 = ml_dtypes.bfloat16

B, S, D = 8, 8192, 512
C = 64                      # chunk size
NCH = S // C                # 128 chunks
P = 128                     # partitions
DT = D // P                 # 4 feature tiles
JT = 512                    # positions per matmul tile
NJ = S // JT                # 16 pos-tiles
HID, NEXP = 128, 3

N_CORES = 8

_CACHE = {}


def _make_pools(ctx, tc):
    return {
        "consts": ctx.enter_context(tc.tile_pool(name="consts", bufs=1)),
        "xtp": ctx.enter_context(tc.tile_pool(name="xtp", bufs=4)),
        "xnp": ctx.enter_context(tc.tile_pool(name="xnp", bufs=2)),
        "bigp": ctx.enter_context(tc.tile_pool(name="bigp", bufs=1)),
        "grids": ctx.enter_context(tc.tile_pool(name="grids", bufs=1)),
        "scratch": ctx.enter_context(tc.tile_pool(name="scratch", bufs=1)),
        "ps_lg": ctx.enter_context(tc.tile_pool(name="ps_lg", bufs=5, space="PSUM")),
        "ps_m": ctx.enter_context(tc.tile_pool(name="ps_m", bufs=1, space="PSUM")),
        "ps_epi": ctx.enter_context(tc.tile_pool(name="ps_epi", bufs=2, space="PSUM")),
    }


def _emit_body(pools, nc, tc, dram, mybir):
    """Emit one full forward pass for one core."""
    f32 = mybir.dt.float32
    bf16 = mybir.dt.bfloat16
    AF = mybir.ActivationFunctionType
    OP = mybir.AluOpType

    consts = pools["consts"]
    xtp = pools["xtp"]
    xnp = pools["xnp"]
    bigp = pools["bigp"]
    grids = pools["grids"]
    scratch = pools["scratch"]
    ps_lg = pools["ps_lg"]
    ps_m = pools["ps_m"]
    ps_epi = pools["ps_epi"]

    # [512, X] dram tensors load as one [128, 4, X] tile each (one DMA).
    def load4(src, cols, dtype, nm):
        t = consts.tile([P, DT, cols], dtype, tag=nm, name=nm)
        nc.sync.dma_start(
            out=t[:], in_=src[:, :].rearrange("(a p) c -> p a c", p=P))
        return t

    # first stream tiles up front so PE can start ASAP
    xt0 = xtp.tile([P, DT, JT], bf16, tag="xt", name="xt0")
    nc.sync.dma_start(
        out=xt0[:],
        in_=dram["xT"][:, 0:JT].rearrange("(a p) c -> p a c", p=P))
    xn0 = xnp.tile([P, 4, D], bf16, tag="xn", name="xn0")
    nc.sync.dma_start(
        out=xn0[:], in_=dram["xn"][0:JT, :].rearrange("(t p) c -> p t c", p=P))

    # streamed-phase constants (before the rest of the stream)
    aw = []
    for k in range(DT):
        t = consts.tile([P, D], bf16, tag=f"aw{k}", name=f"aw{k}")
        nc.sync.dma_start(out=t[:], in_=dram["attn_w"][k * P:(k + 1) * P, :])
        aw.append(t)
    G = consts.tile([P, 2 * P], bf16, tag="G", name="G")
    nc.sync.dma_start(out=G[:], in_=dram["G"][:])

    # grids for segsum results (combined across d-tiles)
    Zc = grids.tile([P, DT, NCH], f32, tag="Zc", name="Zc")[:]
    Nc = grids.tile([P, DT, NCH], f32, tag="Nc", name="Nc")[:]
    m_ps = ps_m.tile([P, D], f32, tag="m_ps", name="m_ps")

    ones11 = consts.tile([1, 1], f32, tag="ones11", name="ones11")
    nc.vector.memset(ones11[:], 1.0)
    ones1p = consts.tile([1, P], f32, tag="ones1p", name="ones1p")
    nc.vector.memset(ones1p[:], 1.0)

    # ---- front work that depends only on host-prepped firsts/lasts ----
    fp4 = load4(dram["fpad"], NCH + 1, f32, "fp4")
    lp4 = load4(dram["lpad"], NCH + 1, f32, "lp4")
    rw14 = load4(dram["router_w1"], HID, f32, "rw14")
    rw1 = [rw14[:, k] for k in range(DT)]
    rb1 = consts.tile([1, HID], f32, tag="rb1", name="rb1")
    nc.sync.dma_start(out=rb1[:], in_=dram["router_b1"][:])
    rw2 = consts.tile([HID, NEXP], f32, tag="rw2", name="rw2")
    nc.sync.dma_start(out=rw2[:], in_=dram["router_w2"][:])
    rb2 = consts.tile([1, NEXP], f32, tag="rb2", name="rb2")
    nc.sync.dma_start(out=rb2[:], in_=dram["router_b2"][:])
    ones11 = consts.tile([1, 1], f32, tag="ones11", name="ones11")
    nc.vector.memset(ones11[:], 1.0)
    ones1p = consts.tile([1, P], f32, tag="ones1p", name="ones1p")
    nc.vector.memset(ones1p[:], 1.0)

    u = grids.tile([P, DT, NCH], f32, tag="u", name="u")
    nc.vector.tensor_tensor(out=u[:], in0=lp4[:, :, 0:NCH],
                            in1=lp4[:, :, 1:NCH + 1], op=OP.subtract)
    v = grids.tile([P, DT, NCH], f32, tag="v", name="v")
    nc.vector.tensor_tensor(out=v[:], in0=fp4[:, :, 1:NCH + 1],
                            in1=fp4[:, :, 0:NCH], op=OP.subtract)

    # router MLP + softmax + broadcast of r (independent of the stream)
    xfs = grids.tile([P, DT], f32, tag="xfs", name="xfs")
    nc.vector.reduce_sum(out=xfs[:], in_=fp4[:, :, 0:NCH],
                         axis=mybir.AxisListType.X)
    xf = grids.tile([P, DT], f32, tag="xf", name="xf")
    nc.scalar.mul(xf[:], xfs[:], 1.0 / NCH)
    ps_h = ps_epi.tile([P, 1], f32, tag="epi", name="epi")
    for k in range(DT):
        nc.tensor.matmul(ps_h[:], rw1[k][:], xf[:, k:k + 1],
                         start=(k == 0), stop=False)
    nc.tensor.matmul(ps_h[:], rb1[:], ones11[:], start=False, stop=True)
    hsb = grids.tile([P, 1], f32, tag="hsb", name="hsb")
    nc.scalar.activation(out=hsb[:], in_=ps_h[:], func=AF.Relu)
    ps_r = ps_epi.tile([1, NEXP], f32, tag="epi", name="epi")
    nc.tensor.matmul(ps_r[:], hsb[:], rw2[:], start=True, stop=False)
    nc.tensor.matmul(ps_r[:], ones11[:], rb2[:], start=False, stop=True)
    rmax = grids.tile([1, 1], f32, tag="rmax", name="rmax")
    nc.vector.reduce_max(out=rmax[:], in_=ps_r[:], axis=mybir.AxisListType.X)
    nrmax = grids.tile([1, 1], f32, tag="nrmax", name="nrmax")
    nc.vector.tensor_scalar_mul(nrmax[:], rmax[:], -1.0)
    er = grids.tile([1, NEXP], f32, tag="er", name="er")
    nc.scalar.activation(out=er[:], in_=ps_r[:], func=AF.Exp, bias=nrmax[:])
    rsum = grids.tile([1, 1], f32, tag="rsum", name="rsum")
    nc.vector.reduce_sum(out=rsum[:], in_=er[:], axis=mybir.AxisListType.X)
    rrec = grids.tile([1, 1], f32, tag="rrec", name="rrec")
    nc.vector.reciprocal(rrec[:], rsum[:])
    rvec = grids.tile([1, NEXP], f32, tag="rvec", name="rvec")
    nc.vector.tensor_scalar_mul(rvec[:], er[:], rrec[:])
    ps_b = ps_epi.tile([P, NEXP], f32, tag="epi", name="epi")
    nc.tensor.matmul(ps_b[:], ones1p[:], rvec[:], start=True, stop=True)
    rb = grids.tile([P, NEXP], f32, tag="rb", name="rb")
    nc.scalar.copy(rb[:], ps_b[:])

    PIECES = [1, 1, 2, 2, 2, 2, 2, 2, 1, 1]  # small pieces: segsum work starts
    assert sum(PIECES) == NJ                 # early and ends with a tiny tail

    NCH_MAX = max(PIECES) * JT // C

    def make_tree_gen(Ep, Pp, ch0, nch):
        """3-op hybrid segsum per tensor (all 4 d-tiles in one op):
        L1+L2 bf16 pair-adds, then fp32 reduce over the remaining 16."""
        for tile4, grid in ((Ep, Zc), (Pp, Nc)):
            view = tile4[:, :, 0:nch * C].rearrange("p a (n c) -> p a n c", c=C)
            s1 = scratch.tile([P, DT, NCH_MAX, C // 2], bf16, tag="s1",
                              name="s1", bufs=2)[:, :, 0:nch]
            nc.vector.tensor_tensor(
                out=s1, in0=view[:, :, :, 0:32], in1=view[:, :, :, 32:64],
                op=OP.add)
            yield
            s2 = scratch.tile([P, DT, NCH_MAX, C // 4], bf16, tag="s2",
                              name="s2", bufs=2)[:, :, 0:nch]
            nc.vector.tensor_tensor(
                out=s2, in0=s1[:, :, :, 0:16], in1=s1[:, :, :, 16:32],
                op=OP.add)
            yield
            nc.vector.reduce_sum(
                out=grid[:, :, ch0:ch0 + nch], in_=s2,
                axis=mybir.AxisListType.X)
            yield

    def emit_mconv():
        # ------- epilogue part A: conv weights, m transpose (overlaps last work)
        wT = {}
        for w in range(3):
            w4 = load4(dram[f"w{w}T"], D, bf16, f"w{w}T4")
            wT[w] = [w4[:, k] for k in range(DT)]
        cbr = consts.tile([1, D], f32, tag="cbr", name="cbr")
        nc.sync.dma_start(out=cbr[:], in_=dram["conv_b_row"][:])
        ident = consts.tile([P, P], f32, tag="ident", name="ident")
        nc.sync.dma_start(out=ident[:], in_=dram["ident"][:])

        # m: PSUM [128 chunks, 512 d] -> SBUF f32 (scaled 1/64) -> transpose
        m_nat = grids.tile([P, D], f32, tag="m_nat", name="m_nat")
        nc.scalar.mul(m_nat[:], m_ps[:], 1.0 / C)
        mT = grids.tile([P, DT, NCH], f32, tag="mT", name="mT")
        for k in range(DT):
            pst = ps_epi.tile([P, P], f32, tag="epi", name="epi")
            nc.tensor.transpose(pst[:], m_nat[:, k * P:(k + 1) * P], ident[:])
            nc.scalar.copy(mT[:, k], pst[:])
        mTb = grids.tile([P, DT, NCH], bf16, tag="mTb", name="mTb")
        nc.scalar.copy(mTb[:], mT[:])

        # a = m + u/64, c = m + v/64  (bf16 for matmul)
        aTb = grids.tile([P, DT, NCH], bf16, tag="aTb", name="aTb")
        nc.vector.scalar_tensor_tensor(
            out=aTb[:], in0=u[:], scalar=1.0 / C, in1=mT[:],
            op0=OP.mult, op1=OP.add)
        cTb = grids.tile([P, DT, NCH], bf16, tag="cTb", name="cTb")
        nc.vector.scalar_tensor_tensor(
            out=cTb[:], in0=v[:], scalar=1.0 / C, in1=mT[:],
            op0=OP.mult, op1=OP.add)

        # conv expert: 12 matmuls + bias matmul, then copy to SBUF
        convT = grids.tile([P, DT, NCH], f32, tag="convT", name="convT")
        for o in range(DT):
            ps = ps_epi.tile([P, NCH], f32, tag="epi", name="epi")
            first = True
            for w, rhs4 in ((0, aTb), (1, mTb), (2, cTb)):
                for k in range(DT):
                    nc.tensor.matmul(
                        ps[:], wT[w][k][:, o * P:(o + 1) * P], rhs4[:, k],
                        start=first, stop=False)
                    first = False
            nc.tensor.matmul(
                ps[:], cbr[:, o * P:(o + 1) * P], ones1p[:],
                start=False, stop=True)
            nc.scalar.copy(convT[:, o], ps[:])

        # r0 * m term of the mix (ready early; ACT per-partition scale)
        tmp = grids.tile([P, DT, NCH], f32, tag="tmp", name="tmp")
        nc.scalar.mul(tmp[:], mT[:], rb[:, 0:1])
        return convT, mT, tmp

    # ---------------- main streaming phase ----------------
    # xn (natural layout) streams ahead of the xT stream so the m chunk-sum
    # matmuls finish early and the m->conv chain hides under the stream.
    pending = None
    jbase = 0
    jn = 0
    xn_cur, xn_tile_idx = xn0, 0

    def gmm_until(limit):
        nonlocal jn, xn_cur, xn_tile_idx
        while jn < min(limit, 4 * NJ):
            t = jn // 4
            if t != xn_tile_idx:
                xn_cur = xnp.tile([P, 4, D], bf16, tag="xn", name="xn")
                nc.sync.dma_start(
                    out=xn_cur[:],
                    in_=dram["xn"][t * JT:(t + 1) * JT, :].rearrange(
                        "(t p) c -> p t c", p=P))
                xn_tile_idx = t
            nc.tensor.matmul(
                m_ps[:], G[:, P - 2 * jn:2 * P - 2 * jn], xn_cur[:, jn % 4],
                start=(jn == 0), stop=(jn == 4 * NJ - 1),
                skip_group_check=True)
            jn += 1

    for pc in range(len(PIECES)):
        PJp = PIECES[pc]
        Ep = bigp.tile([P, DT, max(PIECES) * JT], bf16, tag="Ep", name="Ep",
                       bufs=2)
        Pp = bigp.tile([P, DT, max(PIECES) * JT], bf16, tag="Pp", name="Pp",
                       bufs=2)
        for jj in range(PJp):
            j = jbase + jj
            if j == 0:
                xt = xt0
            else:
                xt = xtp.tile([P, DT, JT], bf16, tag="xt", name="xt")
                nc.sync.dma_start(
                    out=xt[:],
                    in_=dram["xT"][:, j * JT:(j + 1) * JT].rearrange(
                        "(a p) c -> p a c", p=P))
            off = jj * JT
            for o in range(DT):
                ps = ps_lg.tile([P, JT], f32, tag="lg", name="lg")
                for k in range(DT):
                    nc.tensor.matmul(
                        ps[:], aw[k][:, o * P:(o + 1) * P], xt[:, k],
                        start=(k == 0), stop=(k == DT - 1))
                nc.scalar.activation(
                    out=Ep[:, o, off:off + JT], in_=ps[:], func=AF.Exp)
            nc.vector.tensor_tensor(
                out=Pp[:, :, off:off + JT], in0=xt[:],
                in1=Ep[:, :, off:off + JT], op=OP.mult)
            gmm_until(5 * (j + 1))
            # interleave previous piece's segsum ops (3 per pos-tile)
            if pending is not None:
                for _ in range(3):
                    if next(pending, "done") == "done":
                        pending = None
                        break
            if j == 13:
                convT, mT, tmp = emit_mconv()
        if pending is not None:
            for _ in pending:
                pass
        pending = make_tree_gen(Ep, Pp, jbase * JT // C, PJp * JT // C)
        jbase += PJp

    # ------- drain remaining segsum ops (last piece)
    if pending is not None:
        for _ in pending:
            pass

    # attention expert: N / Z
    rz = grids.tile([P, DT, NCH], f32, tag="rz", name="rz")
    nc.vector.reciprocal(rz[:], Zc)
    attnT = grids.tile([P, DT, NCH], f32, tag="attnT", name="attnT")
    nc.vector.tensor_tensor(out=attnT[:], in0=Nc, in1=rz[:], op=OP.mult)

    # mix and write out (single combined DMA)
    acc = grids.tile([P, DT, NCH], f32, tag="acc", name="acc")
    nc.vector.scalar_tensor_tensor(
        out=acc[:], in0=attnT[:], scalar=rb[:, 1:2], in1=tmp[:],
        op0=OP.mult, op1=OP.add)
    y4 = grids.tile([P, DT, NCH], f32, tag="y4", name="y4")
    nc.vector.scalar_tensor_tensor(
        out=y4[:], in0=convT[:], scalar=rb[:, 2:3], in1=acc[:],
        op0=OP.mult, op1=OP.add)
    nc.sync.dma_start(
        out=dram["y"][:, :].rearrange("(a p) n -> p a n", p=P), in_=y4[:])


def _build(loop_iters=None):
    import concourse.bass as bass
    from concourse import bacc
    import concourse.mybir as mybir
    import concourse.tile as tile

    f32 = mybir.dt.float32
    bf16 = mybir.dt.bfloat16

    nc = bacc.Bacc(None, target_bir_lowering=False)
    dram = {
        "xT": nc.dram_tensor("xT", [D, S], bf16, kind="ExternalInput"),
        "xn": nc.dram_tensor("xn", [S, D], bf16, kind="ExternalInput"),
        "attn_w": nc.dram_tensor("attn_w", [D, D], bf16, kind="ExternalInput"),
        "w0T": nc.dram_tensor("w0T", [D, D], bf16, kind="ExternalInput"),
        "w1T": nc.dram_tensor("w1T", [D, D], bf16, kind="ExternalInput"),
        "w2T": nc.dram_tensor("w2T", [D, D], bf16, kind="ExternalInput"),
        "fpad": nc.dram_tensor("fpad", [D, NCH + 1], f32, kind="ExternalInput"),
        "lpad": nc.dram_tensor("lpad", [D, NCH + 1], f32, kind="ExternalInput"),
        "router_w1": nc.dram_tensor("router_w1", [D, HID], f32, kind="ExternalInput"),
        "router_b1": nc.dram_tensor("router_b1", [1, HID], f32, kind="ExternalInput"),
        "router_w2": nc.dram_tensor("router_w2", [HID, NEXP], f32, kind="ExternalInput"),
        "router_b2": nc.dram_tensor("router_b2", [1, NEXP], f32, kind="ExternalInput"),
        "conv_b_row": nc.dram_tensor("conv_b_row", [1, D], f32, kind="ExternalInput"),
        "G": nc.dram_tensor("G", [P, 2 * P], bf16, kind="ExternalInput"),
        "ident": nc.dram_tensor("ident", [P, P], f32, kind="ExternalInput"),
        "y": nc.dram_tensor("y", [D, NCH], f32, kind="ExternalOutput"),
    }
    from contextlib import ExitStack
    with tile.TileContext(nc) as tc:
        with ExitStack() as ctx:
            pools = _make_pools(ctx, tc)
            if loop_iters is None:
                _emit_body(pools, nc, tc, dram, mybir)
            else:
                ET = mybir.EngineType
                with tc.For_i(0, loop_iters, 1,
                              hint_engines=(ET.PE, ET.DVE, ET.Activation, ET.SP)):
                    _emit_body(pools, nc, tc, dram, mybir)
    nc.finalize()
    return nc


def _host_prep(inputs):
    """Build per-core input maps from full inputs."""
    x = np.asarray(inputs["x"], dtype=np.float32)
    attn_w = np.asarray(inputs["attn_w"], dtype=np.float32)
    conv_w = np.asarray(inputs["conv_w"], dtype=np.float32)
    conv_b = np.asarray(inputs["conv_b"], dtype=np.float32)
    rw1 = np.asarray(inputs["router_w1"], dtype=np.float32)
    rb1 = np.asarray(inputs["router_b1"], dtype=np.float32)
    rw2 = np.asarray(inputs["router_w2"], dtype=np.float32)
    rb2 = np.asarray(inputs["router_b2"], dtype=np.float32)

    aw_bf = np.ascontiguousarray(attn_w).astype(BF16)
    w0T = np.ascontiguousarray(conv_w[:, :, 0].T).astype(BF16)
    w1T = np.ascontiguousarray(conv_w[:, :, 1].T).astype(BF16)
    w2T = np.ascontiguousarray(conv_w[:, :, 2].T).astype(BF16)
    G = np.zeros((P, 2 * P), BF16)
    G[0:C, P] = 1.0
    G[C:P, P + 1] = 1.0
    ident = np.eye(P, dtype=np.float32)
    rb1_2d = rb1.reshape(1, HID)
    rb2_2d = rb2.reshape(1, NEXP)
    cb_row = conv_b.reshape(1, D)

    in_maps = []
    for b in range(B):
        xb = x[b]
        F = xb[0::C]            # [NCH, D]
        L = xb[C - 1::C]
        fpad = np.zeros((D, NCH + 1), np.float32)
        fpad[:, 0:NCH] = F.T
        lpad = np.zeros((D, NCH + 1), np.float32)
        lpad[:, 1:NCH + 1] = L.T
        in_maps.append({
            "xT": np.ascontiguousarray(xb.T).astype(BF16),
            "xn": xb.astype(BF16),
            "attn_w": aw_bf,
            "w0T": w0T, "w1T": w1T, "w2T": w2T,
            "fpad": fpad, "lpad": lpad,
            "router_w1": rw1, "router_b1": rb1_2d,
            "router_w2": rw2, "router_b2": rb2_2d,
            "conv_b_row": cb_row, "G": G, "ident": ident,
        })
    return in_maps


def kernel(**inputs):
    from concourse.bass_utils import run_bass_kernel_spmd

    if "nc" not in _CACHE:
        _CACHE["nc"] = _build()
    nc = _CACHE["nc"]
    in_maps = _host_prep(inputs)
    res = run_bass_kernel_spmd(nc, in_maps, list(range(N_CORES)))
    out = np.stack([np.ascontiguousarray(res.results[b]["y"].T)
                    for b in range(B)])
    return out.astype(np.float32)


if __name__ == "__main__":
    rng = np.random.default_rng(0)
    fake = {
        "x": rng.standard_normal((B, S, D), dtype=np.float32),
        "attn_w": rng.standard_normal((D, D), dtype=np.float32) / np.sqrt(D),
        "attn_b": np.zeros(D, np.float32),
        "conv_w": rng.standard_normal((D, D, 3), dtype=np.float32) / np.sqrt(3 * D),
        "conv_b": np.zeros(D, np.float32),
        "router_w1": rng.standard_normal((D, HID), dtype=np.float32) / np.sqrt(D),
        "router_b1": np.zeros(HID, np.float32),
        "router_w2": rng.standard_normal((HID, NEXP), dtype=np.float32) / np.sqrt(HID),
        "router_b2": np.zeros(NEXP, np.float32),
    }
    y = kernel(**fake)
    print("kernel out", y.shape, y.dtype, np.abs(y).max())

